# revision 7
# baseline (speedup 1.0000x reference)
"""Trainium2 Bass kernel for a 2-layer GAT (graph attention network).

v2: destination-node sharding across 8 NeuronCores with a global degree-sorted
node permutation. Edges stored in "transposed CSR" chunks: chunk c of a block
holds edge #c of each of the block's 128 nodes, so the per-chunk aggregation
matmul uses a constant identity stationary operand and attention a_d terms are
read directly per partition. Edge rows are fetched with dma_gather; the two
feature-table halves keep indices in int16 range."""

import sys

sys.path.insert(0, "/opt/trn_rl_repo")
import numpy as np

import concourse.bacc as bacc
import concourse.bass as bass
import concourse.mybir as mybir
from concourse import tile
from concourse.masks import make_identity

F32 = mybir.dt.float32
I16 = mybir.dt.int16
BF16 = mybir.dt.bfloat16
AF = mybir.ActivationFunctionType
ALU = mybir.AluOpType
AX = mybir.AxisListType

NCORES = 8
P = 128
SUPER = 1024
CPS = SUPER // P


def _pad_to(x, m):
    return (x + m - 1) // m * m


def host_preprocess(x, edge_index, W1, att_src1, att_dst1, b1, W2, att_src2, att_dst2, b2):
    x = np.asarray(x, np.float32)
    W1 = np.asarray(W1, np.float32)
    W2 = np.asarray(W2, np.float32)
    att_src1 = np.asarray(att_src1, np.float32)
    att_dst1 = np.asarray(att_dst1, np.float32)
    att_src2 = np.asarray(att_src2, np.float32)
    att_dst2 = np.asarray(att_dst2, np.float32)
    N, IN = x.shape
    HEADS, HID = att_src1.shape
    OUT = att_src2.shape[1]
    C1 = HEADS * HID

    assert N % NCORES == 0
    NPC = N // NCORES
    NBLK = (NPC + P - 1) // P
    NHALF = N // 2

    src0 = np.concatenate([np.asarray(edge_index[0], np.int64), np.arange(N, dtype=np.int64)])
    dst0 = np.concatenate([np.asarray(edge_index[1], np.int64), np.arange(N, dtype=np.int64)])

    # --- global permutation: sort by in-degree, deal round-robin to cores ---
    deg = np.bincount(dst0, minlength=N)
    rank = np.argsort(-deg, kind="stable")
    core_of_node = np.zeros(N, np.int64)
    core_of_node[rank] = np.arange(N) % NCORES
    # int16 index windows, core-aligned with overlap: half A = cores [0,5),
    # half B = cores [3,8). Edges whose src sits in cores 3-4 can use either
    # half; balance them per destination node so block max-degrees shrink.
    A_HI = 5 * NPC
    B_LO = 3 * NPC
    assert A_HI <= 32768 and (N - B_LO) <= 32768
    csrc = core_of_node[src0]
    o0 = np.argsort(dst0, kind="stable")
    d_ = dst0[o0]
    fa_ = (csrc < 3)[o0]
    fb_ = (csrc >= 5)[o0]
    nAf = np.bincount(d_[fa_], minlength=N)
    nBf = np.bincount(d_[fb_], minlength=N)
    nF = deg - nAf - nBf
    tA = np.clip((deg + 1) // 2, nAf, nAf + nF)
    half_sorted = np.where(fa_, 0, np.where(fb_, 1, -1))
    idxf = np.nonzero(half_sorted == -1)[0]
    df = d_[idxf]
    occf = np.arange(len(idxf)) - np.searchsorted(df, df)
    half_sorted[idxf] = np.where(occf < (tA - nAf)[df], 0, 1)
    dA = np.bincount(d_[half_sorted == 0], minlength=N)
    dB = np.bincount(d_[half_sorted == 1], minlength=N)
    # serpentine within-core sort by (dA, +/-dB) to keep blocks degree-uniform
    serp = np.where(dA % 2 == 0, dB, 10**6 - dB)
    perm = np.zeros(N, np.int64)
    for c in range(NCORES):
        nodes = np.nonzero(core_of_node == c)[0]
        order = nodes[np.lexsort((serp[nodes], dA[nodes]))]
        perm[order] = c * NPC + np.arange(NPC)
    inv_perm = np.argsort(perm, kind="stable")

    src_s = perm[src0[o0]]
    dst_s = perm[d_]
    order = np.argsort(dst_s, kind="stable")
    src = src_s[order]
    dst = dst_s[order]
    half_of = half_sorted[order]

    # per-node half-degrees in permuted space
    pdegA = np.zeros(N, np.int64); pdegA[perm] = dA
    pdegB = np.zeros(N, np.int64); pdegB[perm] = dB
    ch_count = np.zeros((NCORES, NBLK, 2), np.int64)
    for c in range(NCORES):
        lo = c * NPC
        for b in range(NBLK):
            nlo = lo + b * P
            nhi = min(lo + (b + 1) * P, lo + NPC)
            ch_count[c, b, 0] = pdegA[nlo:nhi].max()
            ch_count[c, b, 1] = pdegB[nlo:nhi].max()
    ch_common = ch_count.max(axis=0)  # [NBLK, 2]

    streams = {0: [], 1: []}
    for b in range(NBLK):
        for h in (0, 1):
            for c_i in range(int(ch_common[b, h])):
                streams[h].append((b, c_i))
    for h in (0, 1):
        while len(streams[h]) % CPS != 0:
            streams[h].append((-1, 0))
    ns = {h: len(streams[h]) // CPS for h in (0, 1)}

    # merge supers of the two halves keeping block windows tight: always take
    # the stream whose next super starts at the smallest block id
    def _sblk(h, s):
        blocks = [b for b, _ in streams[h][s * CPS : (s + 1) * CPS] if b >= 0]
        return min(blocks) if blocks else 10**9

    emit = []
    i0 = i1 = 0
    while i0 < ns[0] or i1 < ns[1]:
        if i1 >= ns[1] or (i0 < ns[0] and _sblk(0, i0) <= _sblk(1, i1)):
            emit.append((0, i0)); i0 += 1
        else:
            emit.append((1, i1)); i1 += 1

    chunk_seq = []  # (half, block, c_i)
    for h, s in emit:
        for j in range(CPS):
            b, c_i = streams[h][s * CPS + j]
            chunk_seq.append((h, b, c_i))
    NCH = len(chunk_seq)
    first_of_block, last_of_block = {}, {}
    for i, (h, b, c_i) in enumerate(chunk_seq):
        if b < 0:
            continue
        first_of_block.setdefault(b, i)
        last_of_block[b] = i
    flags = [
        (b >= 0 and first_of_block[b] == i, b >= 0 and last_of_block[b] == i)
        for i, (h, b, c_i) in enumerate(chunk_seq)
    ]

    W1W = C1 + 2 * HEADS
    W2C = OUT + 2
    plan = dict(
        N=N, IN=IN, HEADS=HEADS, HID=HID, OUT=OUT, C1=C1, NPC=NPC, NBLK=NBLK,
        NHALF=NHALF, A_HI=A_HI, B_LO=B_LO, emit=emit, chunk_seq=chunk_seq, flags=flags, NCH=NCH,
        W1W=W1W, ROW1=_pad_to((C1 + 2 * HEADS) * 2, 256) // 2,
        W2C=W2C, ROW2=_pad_to(W2C * 4, 256) // 4,
    )

    W1ext = np.zeros((IN, W1W), np.float32)
    W1ext[:, :C1] = W1
    for hh in range(HEADS):
        Wh = W1[:, hh * HID : (hh + 1) * HID]
        W1ext[:, C1 + hh] = Wh @ att_src1[hh]
        W1ext[:, C1 + HEADS + hh] = Wh @ att_dst1[hh]
    W2ext = np.zeros((C1, W2C), np.float32)
    W2ext[:, :OUT] = W2
    W2ext[:, OUT] = W2 @ att_src2[0]
    W2ext[:, OUT + 1] = W2 @ att_dst2[0]
    assert C1 % P == 0
    KC = C1 // P
    W2ext_packed = W2ext.reshape(KC, P, W2C).transpose(1, 2, 0).transpose(0, 2, 1)
    W2ext_packed = W2ext.reshape(KC, P, W2C).transpose(1, 0, 2).reshape(P, KC * W2C)

    # occurrence index of each edge within its (dst, half) group
    keys = dst * 2 + half_of
    sort2 = np.argsort(keys, kind="stable")
    kk = keys[sort2]
    occ_sorted = np.arange(len(src)) - np.searchsorted(kk, kk)
    occ = np.zeros(len(src), np.int64)
    occ[sort2] = occ_sorted

    chunk_idx = {}
    for i, (h, b, c_i) in enumerate(chunk_seq):
        if b >= 0:
            chunk_idx.setdefault((b, h, c_i), i)

    xP = x[inv_perm]
    per_core = []
    for c in range(NCORES):
        lo, hi = c * NPC, (c + 1) * NPC
        m = (dst >= lo) & (dst < hi)
        e_src = src[m]
        e_half = half_of[m]
        e_occ = occ[m]
        loc = dst[m] - lo
        b_arr = loc // P
        p_arr = loc % P
        idx_list = np.zeros((NCH, P), np.int16)
        padbias = np.full((NCH, P), -100.0, np.float32)
        ci_arr = np.fromiter(
            (chunk_idx[(int(b), int(h), int(o))] for b, h, o in zip(b_arr, e_half, e_occ)),
            dtype=np.int64, count=len(e_src),
        )
        idx_list[ci_arr, p_arr] = (e_src - e_half * B_LO).astype(np.int16)
        padbias[ci_arr, p_arr] = 0.0
        idx_w = np.zeros((P, len(emit) * (SUPER // 16)), np.int16)
        for e_i in range(len(emit)):
            flat = idx_list[e_i * CPS : (e_i + 1) * CPS].reshape(SUPER)
            wrapped = flat.reshape(SUPER // 16, 16).T
            idx_w[:, e_i * (SUPER // 16) : (e_i + 1) * (SUPER // 16)] = np.tile(wrapped, (8, 1))
        per_core.append(
            dict(
                xT=np.ascontiguousarray(xP[lo:hi].T),
                W1ext=np.ascontiguousarray(W1ext),
                W2ext=np.ascontiguousarray(W2ext_packed),
                b1r=np.tile(np.asarray(b1, np.float32)[None, :], (P, 1)),
                b2r=np.tile(np.asarray(b2, np.float32)[None, :], (P, 1)),
                idxw=idx_w,
                metaf=np.ascontiguousarray(padbias.T),  # [P, NCH]
            )
        )
    return plan, per_core, perm


def build_kernel(plan):
    N = plan["N"]; IN = plan["IN"]; HEADS = plan["HEADS"]; HID = plan["HID"]
    OUT = plan["OUT"]; C1 = plan["C1"]; NPC = plan["NPC"]; NBLK = plan["NBLK"]
    NHALF = plan["NHALF"]; emit = plan["emit"]; chunk_seq = plan["chunk_seq"]
    A_HI = plan["A_HI"]; B_LO = plan["B_LO"]
    flags = plan["flags"]; NCH = plan["NCH"]
    ROW1 = plan["ROW1"]; ROW2 = plan["ROW2"]; W1W = plan["W1W"]; W2C = plan["W2C"]
    KC = C1 // P
    NSUP = len(emit)
    IW = SUPER // 16

    nc = bacc.Bacc("TRN2", target_bir_lowering=False, debug=False, num_devices=NCORES)

    xT_in = nc.dram_tensor("xT", [IN, NPC], F32, kind="ExternalInput")
    W1e_in = nc.dram_tensor("W1ext", [IN, W1W], F32, kind="ExternalInput")
    W2e_in = nc.dram_tensor("W2ext", [P, KC * W2C], F32, kind="ExternalInput")
    b1r_in = nc.dram_tensor("b1r", [P, C1], F32, kind="ExternalInput")
    b2r_in = nc.dram_tensor("b2r", [P, OUT], F32, kind="ExternalInput")
    idx_in = nc.dram_tensor("idxw", [P, NSUP * IW], I16, kind="ExternalInput")
    metaf_in = nc.dram_tensor("metaf", [P, NCH], F32, kind="ExternalInput")
    out_t = nc.dram_tensor("out", [NPC, OUT], F32, kind="ExternalOutput")

    with tile.TileContext(nc) as tc:
        with (
            tc.tile_pool(name="dram", bufs=1, space="DRAM") as dpool,
            tc.tile_pool(name="const", bufs=1) as cpool,
            tc.tile_pool(name="meta", bufs=1) as mpool,
            tc.tile_pool(name="keep", bufs=1) as kpool,
            tc.tile_pool(name="sm", bufs=6) as smpool,
            tc.tile_pool(name="ep", bufs=2) as eppool,
        ):
            cc1_in = dpool.tile([NPC, ROW1], BF16, name="cc1_in")
            cc1_out = dpool.tile([N, ROW1], BF16, addr_space="Shared", name="cc1_out")
            cc2_in = dpool.tile([NPC, ROW2], F32, name="cc2_in")
            cc2_out = dpool.tile([N, ROW2], F32, addr_space="Shared", name="cc2_out")
            # ---- constants ----
            ident = cpool.tile([P, P], F32)
            make_identity(nc, ident[:])
            identb = cpool.tile([P, P], BF16)
            nc.vector.tensor_copy(out=identb[:], in_=ident[:])
            W1e = cpool.tile([IN, W1W], F32)
            nc.sync.dma_start(out=W1e[:], in_=W1e_in[:])
            W2e = cpool.tile([P, KC * W2C], F32)
            nc.sync.dma_start(out=W2e[:], in_=W2e_in[:])
            b1r = cpool.tile([P, C1], F32)
            nc.sync.dma_start(out=b1r[:], in_=b1r_in[:])
            b2r = cpool.tile([P, OUT], F32)
            nc.sync.dma_start(out=b2r[:], in_=b2r_in[:])
            idxw = mpool.tile([P, NSUP * IW], I16)
            nc.sync.dma_start(out=idxw[:], in_=idx_in[:])
            metaf = mpool.tile([P, NCH], F32)
            nc.sync.dma_start(out=metaf[:], in_=metaf_in[:])
            ad1_keep = kpool.tile([P, NBLK * HEADS], F32)
            nc.vector.memset(ad1_keep[:], 0.0)
            ad2_keep = kpool.tile([P, NBLK], F32)
            nc.vector.memset(ad2_keep[:], 0.0)

            # ---- phase 0 ----
            with (
                tc.tile_pool(name="xT", bufs=1) as xpool,
                tc.tile_pool(name="p0s", bufs=3) as p0s,
                tc.tile_pool(name="ps_p0", bufs=2, space="PSUM") as ps_p0,
            ):
                xT = xpool.tile([IN, NPC], F32)
                nc.sync.dma_start(out=xT[:], in_=xT_in[:])
                for b in range(NBLK):
                    nb = min(P, NPC - b * P)
                    p0 = ps_p0.tile([P, W1W], F32, tag="p0")
                    nc.tensor.matmul(
                        out=p0[:nb, :], lhsT=xT[:, b * P : b * P + nb], rhs=W1e[:],
                        start=True, stop=True,
                    )
                    st = p0s.tile([P, ROW1], BF16, tag="st")
                    nc.vector.tensor_copy(out=st[:nb, :C1], in_=p0[:nb, :C1])
                    nc.vector.tensor_copy(
                        out=st[:nb, C1 : C1 + 2 * HEADS].bitcast(F32),
                        in_=p0[:nb, C1 : C1 + HEADS],
                    )
                    nc.vector.tensor_copy(
                        out=ad1_keep[:nb, b * HEADS : (b + 1) * HEADS],
                        in_=p0[:nb, C1 + HEADS : C1 + 2 * HEADS],
                    )
                    nc.sync.dma_start(
                        out=cc1_in[b * P : b * P + nb, :], in_=st[:nb, :]
                    )
            nc.gpsimd.collective_compute(
                "AllGather", ALU.bypass, replica_groups=[list(range(NCORES))],
                ins=[cc1_in[:]], outs=[cc1_out[:]],
            )

            # ---- phase 1 ----
            with (
                tc.tile_pool(name="g1", bufs=8) as g1pool,
                tc.tile_pool(name="ps_ag1", bufs=6, space="PSUM") as ps_ag,
                tc.tile_pool(name="ps_tr", bufs=2, space="PSUM") as ps_tr,
                tc.tile_pool(name="ps_sm", bufs=1, space="PSUM") as ps_sm,
            ):
                psum_of_block = {}
                for e_i, (hf, s_i) in enumerate(emit):
                    g = g1pool.tile([P, CPS * ROW1], BF16, tag="g1")
                    nc.gpsimd.dma_gather(
                        out_ap=g[:].rearrange("p (k d) -> p k d", d=ROW1),
                        in_ap=(cc1_out[0:A_HI, :] if hf == 0 else cc1_out[B_LO:N, :]),
                        idxs_ap=idxw[:, e_i * IW : (e_i + 1) * IW],
                        num_idxs=SUPER, num_idxs_reg=SUPER, elem_size=ROW1,
                    )
                    for j in range(CPS):
                        ci = e_i * CPS + j
                        _, blk, _c = chunk_seq[ci]
                        if blk < 0:
                            continue
                        isfirst, islast = flags[ci]
                        gc = g[:, j * ROW1 : (j + 1) * ROW1]
                        pbias = metaf[:, ci : ci + 1]
                        if isfirst:
                            psum_of_block[blk] = ps_ag.tile(
                                [P, C1 + HEADS], F32, tag="agg1", name=f"agg1_{blk}"
                            )
                        pb = psum_of_block[blk]
                        t = smpool.tile([P, HEADS], F32, tag="t")
                        nc.vector.tensor_add(
                            out=t[:], in0=gc[:, C1 : C1 + 2 * HEADS].bitcast(F32),
                            in1=ad1_keep[:, blk * HEADS : (blk + 1) * HEADS],
                        )
                        nc.vector.scalar_tensor_tensor(
                            out=t[:], in0=t[:], scalar=0.2, in1=t[:],
                            op0=ALU.mult, op1=ALU.max,
                        )
                        nc.scalar.activation(
                            out=gc[:, C1 : C1 + HEADS], in_=t[:], func=AF.Exp, bias=pbias
                        )
                        nc.vector.tensor_mul(
                            out=gc[:, :C1].rearrange("p (hh d) -> p hh d", hh=HEADS),
                            in0=gc[:, :C1].rearrange("p (hh d) -> p hh d", hh=HEADS),
                            in1=gc[:, C1 : C1 + HEADS].to_broadcast([P, HEADS, HID]),
                        )
                        nc.tensor.matmul(
                            out=pb[:], lhsT=identb[:], rhs=gc[:, : C1 + HEADS],
                            start=isfirst, stop=islast, skip_group_check=True,
                        )
                        if islast:
                            _layer1_epilogue(
                                nc, plan, blk, pb, eppool, ps_tr, ps_sm, smpool,
                                b1r, W2e, ident, ad2_keep, cc2_in,
                            )
                            del psum_of_block[blk]
            nc.gpsimd.collective_compute(
                "AllGather", ALU.bypass, replica_groups=[list(range(NCORES))],
                ins=[cc2_in[:]], outs=[cc2_out[:]],
            )

            # ---- phase 2 ----
            with (
                tc.tile_pool(name="g2", bufs=8) as g2pool,
                tc.tile_pool(name="ps_ag2", bufs=6, space="PSUM") as ps_ag,
            ):
                psum_of_block = {}
                for e_i, (hf, s_i) in enumerate(emit):
                    g = g2pool.tile([P, CPS * ROW2], F32, tag="g2")
                    nc.gpsimd.dma_gather(
                        out_ap=g[:].rearrange("p (k d) -> p k d", d=ROW2),
                        in_ap=(cc2_out[0:A_HI, :] if hf == 0 else cc2_out[B_LO:N, :]),
                        idxs_ap=idxw[:, e_i * IW : (e_i + 1) * IW],
                        num_idxs=SUPER, num_idxs_reg=SUPER, elem_size=ROW2,
                    )
                    for j in range(CPS):
                        ci = e_i * CPS + j
                        _, blk, _c = chunk_seq[ci]
                        if blk < 0:
                            continue
                        isfirst, islast = flags[ci]
                        gc = g[:, j * ROW2 : (j + 1) * ROW2]
                        pbias = metaf[:, ci : ci + 1]
                        if isfirst:
                            psum_of_block[blk] = ps_ag.tile(
                                [P, W2C - 1], F32, tag="agg2", name=f"agg2_{blk}"
                            )
                        pb = psum_of_block[blk]
                        t = smpool.tile([P, 1], F32, tag="t2")
                        nc.vector.tensor_add(
                            out=t[:], in0=gc[:, OUT : OUT + 1],
                            in1=ad2_keep[:, blk : blk + 1],
                        )
                        nc.vector.scalar_tensor_tensor(
                            out=t[:], in0=t[:], scalar=0.2, in1=t[:],
                            op0=ALU.mult, op1=ALU.max,
                        )
                        nc.scalar.activation(
                            out=gc[:, OUT : OUT + 1], in_=t[:], func=AF.Exp, bias=pbias
                        )
                        nc.scalar.mul(
                            out=gc[:, :OUT], in_=gc[:, :OUT], mul=gc[:, OUT : OUT + 1]
                        )
                        nc.tensor.matmul(
                            out=pb[:], lhsT=ident[:], rhs=gc[:, : OUT + 1],
                            start=isfirst, stop=islast, skip_group_check=True,
                        )
                        if islast:
                            _layer2_epilogue(nc, plan, blk, pb, eppool, smpool, b2r, out_t)
                            del psum_of_block[blk]
    nc.compile()
    return nc


def _layer1_epilogue(nc, plan, blk, pb, eppool, ps_tr, ps_sm, smpool, b1r, W2e, ident, ad2_keep, cc2_in):
    HEADS = plan["HEADS"]; HID = plan["HID"]; C1 = plan["C1"]; OUT = plan["OUT"]
    NPC = plan["NPC"]; W2C = plan["W2C"]; KC = C1 // P
    nb = min(P, NPC - blk * P)
    den = smpool.tile([P, HEADS], F32, tag="den")
    nc.vector.tensor_scalar_add(out=den[:], in0=pb[:, C1 : C1 + HEADS], scalar1=1e-16)
    rec = smpool.tile([P, HEADS], F32, tag="rec")
    nc.vector.reciprocal(out=rec[:], in_=den[:])
    h = eppool.tile([P, C1], F32, tag="h")
    nc.vector.tensor_mul(
        out=h[:].rearrange("p (hh d) -> p hh d", hh=HEADS),
        in0=pb[:, :C1].rearrange("p (hh d) -> p hh d", hh=HEADS),
        in1=rec[:].to_broadcast([P, HEADS, HID]),
    )
    nc.vector.tensor_add(out=h[:], in0=h[:], in1=b1r[:])
    hm = eppool.tile([P, C1], F32, tag="hm")
    nc.vector.tensor_scalar_min(out=hm[:], in0=h[:], scalar1=0.0)
    nc.scalar.activation(out=hm[:], in_=hm[:], func=AF.Exp)
    nc.vector.tensor_scalar_max(out=h[:], in0=h[:], scalar1=0.0)
    nc.vector.scalar_tensor_tensor(
        out=h[:], in0=hm[:], scalar=-1.0, in1=h[:], op0=ALU.add, op1=ALU.add
    )
    xp2 = ps_sm.tile([P, W2C], F32, tag="xp2", bufs=1)
    for k in range(KC):
        htp = ps_tr.tile([P, P], F32, tag="htp", bufs=1)
        nc.tensor.transpose(out=htp[:], in_=h[:, k * P : (k + 1) * P], identity=ident[:])
        hT = eppool.tile([P, P], F32, tag="hT")
        nc.vector.tensor_copy(out=hT[:], in_=htp[:])
        nc.tensor.matmul(
            out=xp2[:], lhsT=hT[:], rhs=W2e[:, k * W2C : (k + 1) * W2C],
            start=(k == 0), stop=(k == KC - 1), skip_group_check=True,
        )
    st = eppool.tile([P, W2C], F32, tag="st2")
    nc.vector.tensor_copy(out=st[:nb, :], in_=xp2[:nb, :])
    nc.vector.tensor_copy(
        out=ad2_keep[:nb, blk : blk + 1], in_=xp2[:nb, OUT + 1 : OUT + 2]
    )
    nc.sync.dma_start(out=cc2_in[blk * P : blk * P + nb, :W2C], in_=st[:nb, :])


def _layer2_epilogue(nc, plan, blk, pb, eppool, smpool, b2r, out_t):
    OUT = plan["OUT"]; NPC = plan["NPC"]
    nb = min(P, NPC - blk * P)
    den = smpool.tile([P, 1], F32, tag="den2")
    nc.vector.tensor_scalar_add(out=den[:], in0=pb[:, OUT : OUT + 1], scalar1=1e-16)
    rec = smpool.tile([P, 1], F32, tag="rec2")
    nc.vector.reciprocal(out=rec[:], in_=den[:])
    o = eppool.tile([P, OUT], F32, tag="o")
    nc.vector.tensor_scalar_mul(out=o[:], in0=pb[:, :OUT], scalar1=rec[:])
    nc.vector.tensor_add(out=o[:], in0=o[:], in1=b2r[:])
    mx = smpool.tile([P, 1], F32, tag="mx")
    nc.vector.tensor_reduce(out=mx[:], in_=o[:], axis=AX.X, op=ALU.max)
    nmx = smpool.tile([P, 1], F32, tag="nmx")
    nc.vector.tensor_scalar_mul(out=nmx[:], in0=mx[:], scalar1=-1.0)
    nc.scalar.activation(out=o[:], in_=o[:], func=AF.Exp, bias=nmx[:])
    sm = smpool.tile([P, 1], F32, tag="sm")
    nc.vector.tensor_reduce(out=sm[:], in_=o[:], axis=AX.X, op=ALU.add)
    rs = smpool.tile([P, 1], F32, tag="rs")
    nc.vector.reciprocal(out=rs[:], in_=sm[:])
    nc.vector.tensor_scalar_mul(out=o[:], in0=o[:], scalar1=rs[:])
    nc.sync.dma_start(out=out_t[blk * P : blk * P + nb, :], in_=o[:nb, :])


def run_gat(inputs, trace=False):
    if trace and "antenv.axon_hooks" not in sys.modules:
        try:
            import types as _types

            import trn_agent_boot.trn_boot as _tb

            _mod = _types.ModuleType("antenv.axon_hooks")
            _hook = _tb._ntff_profile_via_ctypes("/opt/axon/libaxon_pjrt.so")
            _mod.get_axon_ntff_profile_hook = lambda: _hook
            sys.modules["antenv.axon_hooks"] = _mod
        except Exception:
            trace = False
    from concourse.bass_utils import run_bass_kernel_spmd

    plan, per_core, perm = host_preprocess(**inputs)
    nc = build_kernel(plan)
    res = run_bass_kernel_spmd(nc, per_core, core_ids=list(range(NCORES)), trace=trace)
    out_p = np.concatenate([res.results[c]["out"] for c in range(NCORES)], axis=0)
    return out_p[perm], res


def kernel(**inputs):
    out, _ = run_gat(inputs, trace=False)
    return out
